# revision 13
# baseline (speedup 1.0000x reference)
"""Trainium2 kernel for ApplyStickerLayer: out = roll(subimg, (80,80), (2,3)) + base_image.

Structure (guaranteed by the layer): subimg is zero outside the 50x50 sticker
at the origin, base_image is zero inside the destination window, and the roll
never wraps -- so per (b, c) channel image (flat, 50176 elems):

    out[bc, f] = base[bc % 3, f] + sub[bc, f - 18000]     (sub oob -> 0)

HW findings driving this design (measured on this part):
  * SBUF AXI port coverage is king: partition p is wired to one of 16 ports.
    DMAs sourcing from partitions 0..15 get ~2 ports (~50 GB/s); partitions
    32..127 cover ALL 16 ports.  So every store sources from a [128, f] tile
    using rows 32..127.
  * SWDGE descriptors cost ~0.7 us fixed (HBM write round trip), so
    descriptors are fat: one 44.8 KB run per (bc) image column chunk.
  * Cross-partition broadcast is only cheap on TensorE: one matmul per column
    chunk replicates base into all 96 images and adds the shifted sticker:

        psum[128, f] = W.T @ x    W [99, 128] = [identity ; channel selector]
                                  x [99, f]   = [96 sub rows ; 3 base rows]

    (columns 0..31 of W are zero; psum rows 32..127 hold images 0..95).
    Pure-base chunks (f outside [18000, 29200)) use only the 3 selector rows.
  * Inputs are cast f32->bf16 during the load DMA; the matmul accumulates in
    f32.  bf16 rounding (~0.4% rel) is far inside the 2e-2 gate.

DVE drains PSUM to SBUF f32 tiles; SWDGE stores them as 96 fat descriptors
per chunk.  Per core ~19.3 MB written + ~4.9 MB read.
"""

import sys

import numpy as np

if "/opt/trn_rl_repo" not in sys.path:
    sys.path.insert(0, "/opt/trn_rl_repo")

import concourse.bacc as bacc
import concourse.bass as bass
import concourse.mybir as mybir
import concourse.tile as tile
from concourse.bass_utils import run_bass_kernel_spmd

N_CORES = 8
B, C, H, W = 256, 3, 224, 224
BS = B // N_CORES  # 32 batches per core
BC = BS * C  # 96 channel images per core
SH, SW = 80, 80
KH, KW = 50, 50

CHW = H * W  # 50176
IMG = C * CHW  # 150528
SHIFT = SH * W + SW  # 18000: the roll as a flat shift
SUB_LEN = (KH - 1) * W + KW + (W - KW)  # 11200: sub cols that can be nonzero
# shifted-sub support inside a channel image: [SHIFT, SHIFT + SUB_LEN)

K = BC + C  # 99: matmul contraction (96 sub rows + 3 base rows)

_F32 = mybir.dt.float32
_BF16 = mybir.dt.bfloat16

DEFAULT_CFG = {
    "fc": 5600,  # column chunk (also the store descriptor length / 4)
    "mm_f": 512,  # matmul free-dim chunk (<= 512, one PSUM bank)
    "psum_bufs": 8,
    "out_bufs": 4,
    "x_bufs": 4,
    "xb_bufs": 4,
    "act_every": 2,  # every act_every-th PSUM evac goes to ScalarE (ACT)
    "store_engs": ("sync", "gpsimd"),  # stores alternate across these rings
    "swq": 1,  # num_swdge_queues
}


def build_nc(cfg=None):
    cfg = {**DEFAULT_CFG, **(cfg or {})}
    fc_max = cfg["fc"]
    mm_f = cfg["mm_f"]

    nc = bacc.Bacc(
        "TRN2",
        target_bir_lowering=False,
        num_devices=N_CORES,
        num_swdge_queues=cfg["swq"],
    )
    sub = nc.declare_dram_parameter("subimg", [BS, C, H, W], _F32, isOutput=False)
    base = nc.declare_dram_parameter("base", [C, H, W], _F32, isOutput=False)
    wsel = nc.declare_dram_parameter("wsel", [K, 128], _F32, isOutput=False)
    out = nc.declare_dram_parameter("out", [BS, C, H, W], _F32, isOutput=True)

    chunks = []
    c0 = 0
    while c0 < CHW:
        chunks.append((c0, min(fc_max, CHW - c0)))
        c0 += fc_max

    with tile.TileContext(nc) as tc:
        with (
            tc.tile_pool(name="consts", bufs=1) as cpool,
            tc.tile_pool(name="work", bufs=1) as wpool,
            tc.tile_pool(name="psum", bufs=cfg["psum_bufs"], space=bass.MemorySpace.PSUM) as ppool,
        ):
            # 128-wide weights: full-width LDWEIGHTS is ~2x faster than 96
            t_wk = cpool.tile([K, 128], _BF16, tag="wk")
            nc.gpsimd.dma_start(out=t_wk[:, :], in_=wsel[:, :])
            t_w3 = cpool.tile([C, 128], _BF16, tag="w3")
            nc.gpsimd.dma_start(out=t_w3[:, :], in_=wsel[BC:K, :])

            for ci, (c0, fc) in enumerate(chunks):
                # sub columns contributing to out cols [c0, c0+fc):
                # sub j = f - SHIFT clipped to [0, SUB_LEN)
                s_lo = max(0, c0 - SHIFT)
                s_hi = min(SUB_LEN, c0 + fc - SHIFT)
                has_sub = s_hi > s_lo

                if has_sub:
                    t_x = wpool.tile([K, fc_max], _BF16, tag="x", bufs=cfg["x_bufs"])
                    x_lo = s_lo + SHIFT - c0  # x column where sub j = s_lo lands
                    x_hi = x_lo + (s_hi - s_lo)
                    if x_lo > 0:
                        nc.vector.memset(t_x[0:BC, 0:x_lo], 0.0)
                    if x_hi < fc:
                        nc.vector.memset(t_x[0:BC, x_hi:fc], 0.0)
                    nc.gpsimd.dma_start(
                        out=t_x[0:BC, x_lo:x_hi],
                        in_=bass.AP(sub, s_lo, [[CHW, BC], [1, s_hi - s_lo]]),
                    )
                    nc.gpsimd.dma_start(
                        out=t_x[BC:K, 0:fc],
                        in_=bass.AP(base, c0, [[CHW, C], [1, fc]]),
                    )
                else:
                    t_x = wpool.tile([C, fc_max], _BF16, tag="xb", bufs=cfg["xb_bufs"])
                    nc.gpsimd.dma_start(
                        out=t_x[0:C, 0:fc],
                        in_=bass.AP(base, c0, [[CHW, C], [1, fc]]),
                    )

                t_o = wpool.tile([BC, fc_max], _F32, tag="out", bufs=cfg["out_bufs"])
                for mi, m0 in enumerate(range(0, fc, mm_f)):
                    mf = min(mm_f, fc - m0)
                    t_p = ppool.tile([128, mm_f], _F32, tag="psum")
                    if has_sub:
                        nc.tensor.matmul(
                            t_p[:, 0:mf], t_wk[:, :], t_x[:, m0 : m0 + mf]
                        )
                    else:
                        nc.tensor.matmul(
                            t_p[:, 0:mf], t_w3[:, :], t_x[0:C, m0 : m0 + mf]
                        )
                    # PSUM drain: mostly DVE, a slice to the idle ACT engine
                    if (mi % cfg["act_every"]) == cfg["act_every"] - 1:
                        nc.scalar.copy(t_o[:, m0 : m0 + mf], t_p[0:BC, 0:mf])
                    else:
                        nc.vector.tensor_copy(t_o[:, m0 : m0 + mf], t_p[0:BC, 0:mf])
                engs = cfg["store_engs"]
                store_eng = getattr(nc, engs[ci % len(engs)])
                store_eng.dma_start(
                    out=bass.AP(out, c0, [[CHW, BC], [1, fc]]),
                    in_=t_o[:, 0:fc],
                )
    nc.compile()
    return nc


def _make_wsel():
    w = np.zeros((K, 128), dtype=np.float32)
    for bc in range(BC):
        w[bc, bc] = 1.0  # identity for the shifted sub rows
        w[BC + bc % C, bc] = 1.0  # base channel selector
    return w


def run(inputs, cfg=None, trace=False, **kw):
    sub = np.ascontiguousarray(inputs["subimg"], dtype=np.float32)
    basei = np.ascontiguousarray(inputs["base_image"], dtype=np.float32)
    assert sub.shape == (B, C, H, W) and basei.shape == (1, C, H, W)

    nc = build_nc(cfg)
    w = _make_wsel()
    in_maps = [
        {"subimg": sub[i * BS : (i + 1) * BS], "base": basei[0], "wsel": w}
        for i in range(N_CORES)
    ]
    res = run_bass_kernel_spmd(nc, in_maps, list(range(N_CORES)), trace=trace, **kw)
    full = np.concatenate(
        [res.results[i]["out"] for i in range(N_CORES)], axis=0
    ).astype(np.float32, copy=False)
    return full, res


def kernel(**inputs) -> np.ndarray:
    out, _ = run(inputs)
    return out
